# revision 1
# baseline (speedup 1.0000x reference)
"""KKAN Convolutional Network kernel for 8 Trainium2 NeuronCores.

Strategy: pure data parallel over batch (32 images -> 4 per core), per
spec sharding hint. The KAN conv is reformulated as a pointwise feature
expansion (silu + 8 cubic B-spline bases per pixel, shared across all
taps/convs) followed by a dense 3x3 conv with 9 input channels and 16
output channels, then the 3x3 restore conv. Both convs run on the PE
array via lax.conv; the basis recursion is elementwise engine work.
"""
import numpy as np
import jax
import jax.numpy as jnp
from functools import partial

GRID_SIZE = 5
SPLINE_ORDER = 3
N_CONVS = 16
K = 3
P = K * K
G = GRID_SIZE + SPLINE_ORDER  # 8
N_CORES = 8
B, H, W = 32, 256, 256


def _grid():
    h = 2.0 / GRID_SIZE
    return np.arange(-SPLINE_ORDER, GRID_SIZE + SPLINE_ORDER + 1, dtype=np.float32) * h - 1.0


def _bases_per_pixel(x):
    # x: (n, 1, H, W) -> (n, G, H, W) cubic B-spline bases, Cox-de Boor
    grid = _grid()
    xx = x  # (n,1,H,W)
    bases = jnp.concatenate(
        [((xx >= grid[i]) & (xx < grid[i + 1])).astype(jnp.float32)
         for i in range(len(grid) - 1)], axis=1)  # (n, 11, H, W)
    for k in range(1, SPLINE_ORDER + 1):
        nb = bases.shape[1] - 1
        left_t = [(xx[:, 0] - grid[i]) / (grid[i + k] - grid[i]) * bases[:, i]
                  for i in range(nb)]
        right_t = [(grid[i + k + 1] - xx[:, 0]) / (grid[i + k + 1] - grid[i + 1]) * bases[:, i + 1]
                   for i in range(nb)]
        bases = jnp.stack([l + r for l, r in zip(left_t, right_t)], axis=1)
    return bases  # (n, G, H, W)


@partial(jax.pmap, in_axes=(0, None, None, None), devices=jax.devices()[:N_CORES])
def _run_shard(x, w1, rw, rb):
    # x: (n,1,H,W); w1: (16, 1+G, 3, 3); rw: (1,16,3,3); rb: (1,)
    sil = jax.nn.silu(x)  # (n,1,H,W)
    bas = _bases_per_pixel(x)  # (n,G,H,W)
    feats = jnp.concatenate([sil, bas], axis=1)  # (n, 9ch, H, W)
    feat = jax.lax.conv_general_dilated(
        feats, w1, (1, 1), [(1, 1), (1, 1)],
        dimension_numbers=('NCHW', 'OIHW', 'NCHW'))  # (n,16,H,W)
    y = jax.lax.conv_general_dilated(
        feat, rw, (1, 1), [(1, 1), (1, 1)],
        dimension_numbers=('NCHW', 'OIHW', 'NCHW'))
    return y + rb[None, :, None, None]


def kernel(x, base_w, spline_w, spline_scaler, restore_w, restore_b):
    x = np.asarray(x, np.float32)
    # Fold base weights + scaled spline weights into one (16, 1+G, 3, 3)
    # conv kernel over the per-pixel feature channels [silu, b_0..b_7].
    sw = (np.asarray(spline_w) * np.asarray(spline_scaler)[..., None]).astype(np.float32)
    w1 = np.zeros((N_CONVS, 1 + G, K, K), np.float32)
    bw = np.asarray(base_w, np.float32)
    for di in range(K):
        for dj in range(K):
            p = di * K + dj
            w1[:, 0, di, dj] = bw[:, p]
            w1[:, 1:, di, dj] = sw[:, p, :]
    xs = x.reshape(N_CORES, B // N_CORES, 1, H, W)
    y = _run_shard(xs, jnp.asarray(w1), jnp.asarray(restore_w, np.float32),
                   jnp.asarray(restore_b, np.float32))
    return np.asarray(y).reshape(B, 1, H, W)

